# revision 14
# baseline (speedup 1.0000x reference)
"""Fused MHA block (QKV + softmax + out-proj + residual + LayerNorm) for
Trainium2, SPMD over 8 NeuronCores — v2 "streaming" structure.

Sharding: head-parallel attention (core c owns heads {2c, 2c+1} for both
batches) with PER-UNIT output exchange. The 4 attention units are
(b, h2) = query blocks of 1024 tokens; each unit is processed as two
512-query half-units (s) so PSUM fits a double-buffered S ring. After each
unit, an 8-way AllToAll exchanges that unit's normalized head outputs so
core c receives ALL 1024 channels for its 128 tokens of the unit
(tokens [1024*h2 + 128*c, +128) of batch b); the unit's out-projection +
residual + LayerNorm then run as filler inside the next unit's exp chain.

Critical path = the ACT exp chain (16.8M exp elems/core, ~1.14us per
[128,1024] activation): everything else (QKV production, S/V matmuls,
collectives, out-proj, LN) is scheduled into its slack. Structure:
  - xT is streamed column-block-major so the first exp can issue ~5us in.
  - S^T layout (keys on partitions): softmax denominator = extra ones
    column in V (M=65 matmuls). exp scale 1/sqrt(64) folded into ACT.
  - S matmul pairs (K=64) land in distinct PE row-groups (tile_position
    auto-derived) -> concurrent on HW.
  - denominator broadcast via K=1 ones-matmul into the just-freed O bank.
  - LN rsqrt = exp(-0.5*ln(var+eps)): Ln+Exp live in the same ACT table
    set (natural_log_exp_and_others) -> no table swaps anywhere.
  - LN normalize runs on DVE (tensor_scalar), not ACT.
PSUM budget: S ring 2x[128,1024] (4 banks) + O/rb 2x[65,512] (2 banks) +
shared qk/v/out-proj ring 2x[128,512] (2 banks) = 8 banks exactly.
"""

import sys

sys.path.insert(0, "/opt/trn_rl_repo")

import numpy as np
import ml_dtypes

BF16 = ml_dtypes.bfloat16

B, T, D = 2, 2048, 1024
H, DH = 16, 64
N_CORES = 8
LN_EPS = 1e-5
HEADS_PER_CORE = 2
TROWS = T * B // N_CORES  # 512 output rows per core
NCH = 8  # 1024 / 128 contraction chunks
N_UNITS = 4  # (b, h2) query blocks

_CACHE = {}

# single-blob element offsets (everything bf16; per-input dispatch cost
# ~14us/tensor through the axon tunnel, so one arg is optimal)
OFF_XT = 0                              # 2 x [D, T]
OFF_WQ = OFF_XT + B * D * T
OFF_WK = OFF_WQ + D * 128
OFF_WV = OFF_WK + D * 128
OFF_WO = OFF_WV + D * 128
OFF_BQ = OFF_WO + D * D
OFF_BK = OFF_BQ + 128
OFF_BV = OFF_BK + 128
OFF_GAMMA = OFF_BV + 128
OFF_BETA = OFF_GAMMA + D
OFF_XRES = OFF_BETA + D                 # 512 rows, unit-major, bo folded in
BF16_TOTAL = OFF_XRES + TROWS * D


def _build(repeat=1, out_bf16=True):
    import os
    from contextlib import ExitStack
    import concourse.bass as bass
    import concourse.tile as tile
    from concourse import bacc, mybir

    import bass_rust
    _dep = bass_rust.add_dep_helper

    f32 = mybir.dt.float32
    bf16 = mybir.dt.bfloat16
    AF = mybir.ActivationFunctionType
    ALU = mybir.AluOpType

    def bcast(ap_src, parts):
        """Broadcast a 1-D (or row) AP across `parts` partitions (step 0)."""
        return bass.AP(tensor=ap_src.tensor, offset=ap_src.offset,
                       ap=[[0, parts]] + [list(p) for p in ap_src.ap])

    nc = bacc.Bacc("TRN2", target_bir_lowering=False, debug=False,
                   num_devices=N_CORES)

    b16 = nc.dram_tensor("blob16", [BF16_TOTAL], bf16, kind="ExternalInput")
    xT_d = [b16[OFF_XT + b * D * T:OFF_XT + (b + 1) * D * T]
            .rearrange("(c t) -> c t", t=T) for b in range(B)]
    wqT_d = b16[OFF_WQ:OFF_WQ + D * 128].rearrange("(c d) -> c d", d=128)
    wkT_d = b16[OFF_WK:OFF_WK + D * 128].rearrange("(c d) -> c d", d=128)
    wvT_d = b16[OFF_WV:OFF_WV + D * 128].rearrange("(c d) -> c d", d=128)
    woT_d = b16[OFF_WO:OFF_WO + D * D].rearrange("(c d) -> c d", d=D)
    bq_d = b16[OFF_BQ:OFF_BQ + 128]
    bk_d = b16[OFF_BK:OFF_BK + 128]
    bv_d = b16[OFF_BV:OFF_BV + 128]
    gamma_d = b16[OFF_GAMMA:OFF_GAMMA + D]
    beta_d = b16[OFF_BETA:OFF_BETA + D]
    xres_d = b16[OFF_XRES:OFF_XRES + TROWS * D].rearrange(
        "(r d) -> r d", d=D)
    out_dt = bf16 if out_bf16 else f32
    out_d = nc.dram_tensor("out", [TROWS, D], out_dt, kind="ExternalOutput")

    def _emit_body(tc):
        ctx = ExitStack()
        persist = ctx.enter_context(tc.tile_pool(name="persist", bufs=1))
        dram = ctx.enter_context(tc.tile_pool(name="dram", bufs=1,
                                              space="DRAM"))

        # ---- warm the natural_log_exp table set (Ln first narrows the
        # chosen set to one containing BOTH ln and exp; LN's rsqrt is
        # exp(-0.5*ln(v)) so no table swap ever happens) ----
        warm = persist.tile([1, 1], f32)
        nc.vector.memset(warm[:], 1.0)
        nc.scalar.activation(warm[:], warm[:], AF.Ln)
        nc.scalar.activation(warm[:], warm[:], AF.Exp, scale=0.125)

        # ---- persistent SBUF tiles ----
        xT_sb = [persist.tile([128, NCH, T], bf16, name=f"xT{b}sb")
                 for b in range(B)]
        wqT_sb = persist.tile([128, NCH, 128], bf16)
        wkT_sb = persist.tile([128, NCH, 128], bf16)
        wvT_sb = persist.tile([128, NCH, 128], bf16)
        bq_sb = persist.tile([128, 1], bf16)
        bk_sb = persist.tile([128, 1], bf16)
        bq_f = persist.tile([128, 1], f32)
        bk_f = persist.tile([128, 1], f32)
        bvb_sb = persist.tile([128, 128], bf16)  # bv bcast across partitions
        woT_sb = persist.tile([128, NCH, D], bf16)
        ones64_sb = persist.tile([1, 64], bf16)
        gamma_sb = persist.tile([128, D], bf16)
        beta_sb = persist.tile([128, D], bf16)
        eps_sb = persist.tile([128, 1], f32)
        xres_sb = persist.tile([128, N_UNITS, D], bf16)
        QT_sb = [persist.tile([128, T], bf16, name=f"QT{b}") for b in range(B)]
        KT_sb = [persist.tile([128, T], bf16, name=f"KT{b}") for b in range(B)]
        # V token-major with ones column per head: [t-tile][128, head, 65]
        V_sb = [[persist.tile([128, HEADS_PER_CORE, DH + 1], bf16,
                              name=f"V{b}_{tt}") for tt in range(16)]
                for b in range(B)]

        nc.vector.memset(ones64_sb[:], 1.0)
        nc.vector.memset(eps_sb[:], LN_EPS)
        for b in range(B):
            for tt in range(16):
                nc.vector.memset(V_sb[b][tt][:, :, DH:DH + 1], 1.0)

        # ---- DMA schedule ----
        # xT b0 n=0 column block first (gates the first S matmuls); rest of
        # b0 next; b1 during units 0-1. sync+scalar pre-start, sync+pool
        # after (scalar must stay free for the exp chain).
        def dma_xt(b, ci, n, eng):
            eng.dma_start(
                out=xT_sb[b][:, ci, 512 * n:512 * (n + 1)],
                in_=xT_d[b][128 * ci:128 * (ci + 1), 512 * n:512 * (n + 1)])

        for ci in range(NCH):
            dma_xt(0, ci, 0, nc.sync if ci % 2 == 0 else nc.scalar)
        nc.gpsimd.dma_start(out=bq_sb[:],
                            in_=bq_d.rearrange("(p f) -> p f", f=1))
        nc.gpsimd.dma_start(out=bk_sb[:],
                            in_=bk_d.rearrange("(p f) -> p f", f=1))
        nc.gpsimd.dma_start(out=wqT_sb[:],
                            in_=wqT_d.rearrange("(ci p) d -> p ci d", p=128))
        nc.gpsimd.dma_start(out=wkT_sb[:],
                            in_=wkT_d.rearrange("(ci p) d -> p ci d", p=128))
        nc.vector.tensor_copy(bq_f[:], bq_sb[:])
        nc.vector.tensor_copy(bk_f[:], bk_sb[:])
        nc.gpsimd.dma_start(out=wvT_sb[:],
                            in_=wvT_d.rearrange("(ci p) d -> p ci d", p=128))
        nc.gpsimd.dma_start(out=bvb_sb[:], in_=bcast(bv_d, 128))
        for n in range(1, 4):
            for ci in range(NCH):
                dma_xt(0, ci, n, nc.sync if ci % 2 == 0 else nc.scalar)
        for n in range(4):
            for ci in range(NCH):
                dma_xt(1, ci, n, nc.sync if ci % 2 == 0 else nc.gpsimd)
        for ci in range(NCH):
            nc.gpsimd.dma_start(out=woT_sb[:, ci, :],
                                in_=woT_d[128 * ci:128 * (ci + 1), :])
        nc.gpsimd.dma_start(out=gamma_sb[:], in_=bcast(gamma_d, 128))
        nc.gpsimd.dma_start(out=beta_sb[:], in_=bcast(beta_d, 128))
        for u in range(N_UNITS):
            nc.gpsimd.dma_start(out=xres_sb[:, u, :],
                                in_=xres_d[128 * u:128 * (u + 1), :])

        # ---- per-unit AllToAll DRAM buffers ----
        a2a_in = [dram.tile([N_CORES, 128, 128], bf16, name=f"a2ai{u}",
                            tag=f"ai{u}") for u in range(N_UNITS)]
        a2a_out = [dram.tile([N_CORES, 128, 128], bf16, name=f"a2ao{u}",
                             tag=f"ao{u}") for u in range(N_UNITS)]

        # ---- pools live for the whole attention+epilogue stream ----
        att_s = ctx.enter_context(tc.tile_pool(name="att_s", bufs=2,
                                               space="PSUM"))
        att_o = ctx.enter_context(tc.tile_pool(name="att_o", bufs=1,
                                               space="PSUM"))
        aux = ctx.enter_context(tc.tile_pool(name="aux", bufs=2,
                                             space="PSUM"))
        pp = ctx.enter_context(tc.tile_pool(name="pp", bufs=3))
        npool = ctx.enter_context(tc.tile_pool(name="npool", bufs=2))
        ogp = ctx.enter_context(tc.tile_pool(name="ogp", bufs=2))
        ln = ctx.enter_context(tc.tile_pool(name="ln", bufs=2))

        # ---- filler emitters (QKV production / out-proj+LN), called
        # between tk iterations so their PE/DVE work lands in exp slack ----
        def emit_qk(b, n, w_sb, bias_sb, dst, half=None, ps=None,
                    ci0=0, ci1=NCH):
            if ps is None:
                ps = aux.tile([128, 512], f32, tag="aux")
            for ci in range(ci0, ci1):
                nc.tensor.matmul(ps[:], w_sb[:, ci, :],
                                 xT_sb[b][:, ci, 512 * n:512 * (n + 1)],
                                 start=(ci == 0), stop=(ci == NCH - 1),
                                 skip_group_check=True)
            if ci1 == NCH:
                nc.vector.tensor_scalar(dst[:, 512 * n:512 * (n + 1)],
                                        ps[:], bias_sb[:], None, ALU.add)
            return ps

        def emit_v(b, tt):
            ps = aux.tile([128, 512], f32, tag="aux")
            for ci in range(NCH):
                nc.tensor.matmul(ps[:, 0:128],
                                 xT_sb[b][:, ci, 128 * tt:128 * (tt + 1)],
                                 wvT_sb[:, ci, :],
                                 start=(ci == 0), stop=(ci == NCH - 1))
            nc.vector.tensor_add(
                V_sb[b][tt][:, :, 0:DH],
                ps[:, 0:128].rearrange("p (h d) -> p h d", h=HEADS_PER_CORE),
                bvb_sb[:].rearrange("p (h d) -> p h d", h=HEADS_PER_CORE))

        def emit_proj_oc(u, oc, y_sb):
            og_sb = _OG[u]
            ps = aux.tile([128, 512], f32, tag="aux")
            for g in range(N_CORES):
                nc.tensor.matmul(ps[:], og_sb[:, g, :],
                                 woT_sb[:, g, 512 * oc:512 * (oc + 1)],
                                 start=(g == 0), stop=(g == N_CORES - 1),
                                 skip_group_check=True)
            nc.vector.tensor_add(
                y_sb[:, 512 * oc:512 * (oc + 1)], ps[:],
                xres_sb[:, u, 512 * oc:512 * (oc + 1)])
            stats = _STATS[u]
            nc.vector.bn_stats(out=stats[:, oc, :],
                               in_=y_sb[:, 512 * oc:512 * (oc + 1)])

        def emit_ln(u, y_sb, final=False):
            stats = _STATS[u]
            mv = ln.tile([128, nc.vector.BN_AGGR_DIM], f32, tag="mv")
            nc.vector.bn_aggr(out=mv[:], in_=stats[:])
            # rstd = exp(-0.5*ln(var+eps)) — same ACT table set as exp
            lnv = ln.tile([128, 1], f32, tag="lnv")
            nc.scalar.activation(lnv[:], mv[:, 1:2], AF.Ln, bias=eps_sb[:])
            rstd = ln.tile([128, 1], f32, tag="rstd")
            nc.scalar.activation(rstd[:], lnv[:], AF.Exp, scale=-0.5)
            negmr = ln.tile([128, 1], f32, tag="negmr")
            nc.vector.tensor_scalar(negmr[:], mv[:, 0:1], rstd[:], -1.0,
                                    ALU.mult, ALU.mult)
            yn = ln.tile([128, D], bf16, tag="yn")
            nc.vector.tensor_scalar(yn[:], y_sb[:], rstd[:], negmr[:],
                                    ALU.mult, ALU.add)
            fing = ln.tile([128, D], bf16, tag="fing")
            nc.vector.tensor_mul(fing[:], yn[:], gamma_sb[:])
            fin = ln.tile([128, D], out_dt, tag="fin")
            if final:
                nc.vector.tensor_add(fin[:], fing[:], beta_sb[:])
            else:
                nc.gpsimd.tensor_add(fin[:], fing[:], beta_sb[:])
            for oh in range(2):
                nc.sync.dma_start(
                    out=out_d[128 * u:128 * (u + 1),
                              512 * oh:512 * (oh + 1)],
                    in_=fin[:, 512 * oh:512 * (oh + 1)])

        _STATS = [None] * N_UNITS

        def OP_(u, final=False):
            state = {}

            def a():
                state["y"] = ln.tile([128, D], f32, tag="y", name=f"y{u}")
                _STATS[u] = ln.tile([128, 2, nc.vector.BN_STATS_DIM], f32,
                                    tag="stats", name=f"st{u}")
                emit_proj_oc(u, 0, state["y"])

            def bql():
                emit_proj_oc(u, 1, state["y"])

            def c():
                emit_ln(u, state["y"], final=final)
            return [a, bql, c]

        def emit_og(u, final=False):
            og_sb = ogp.tile([128, N_CORES, 128], bf16, tag="og",
                             name=f"og{u}")
            _OG[u] = og_sb
            engs = ((nc.sync, nc.scalar, nc.gpsimd) if final
                    else (nc.sync, nc.gpsimd))
            for i in range(N_CORES):
                d = engs[i % len(engs)].dma_start(out=og_sb[:, i, :],
                                                  in_=a2a_out[u][i])
                _dep(d.ins, _CC[u].ins, sync=True,
                     reason="og gather waits a2a")

        _OG = [None] * N_UNITS
        _A2A_DMAS = [[] for _ in range(N_UNITS)]
        _CC = [None] * N_UNITS

        # filler schedule: fillers[hu] = list of closures to interleave into
        # half-unit hu's tk loop (hu = 2*u + s, 8 halves total; [8] = final
        # epilogue). K(b,n) must complete before S(tk=4n); Q(b,n) before the
        # half that reads it; V(b,tt) only gates the O accumulation, so it
        # may lag. Each half's normalize tail is deferred into the next
        # half's first filler slot so it never stalls the exp chain.
        fillers = [[] for _ in range(9)]

        def K_(b, n):
            return lambda: emit_qk(b, n, wkT_sb, bk_f, KT_sb[b])

        def Q_(b, n):
            return lambda: emit_qk(b, n, wqT_sb, bq_f, QT_sb[b])

        def V_(b, tt):
            return lambda: emit_v(b, tt)

        fillers[0] = [K_(0, 1), V_(0, 4), V_(0, 5), K_(0, 2), V_(0, 6),
                      V_(0, 7), V_(0, 8), K_(0, 3), V_(0, 9), V_(0, 10),
                      V_(0, 11), Q_(0, 1), V_(0, 12), V_(0, 13), V_(0, 14),
                      V_(0, 15)]
        fillers[1] = [Q_(0, 2), Q_(0, 3), K_(1, 0)]
        fillers[2] = [K_(1, 1), V_(1, 0), V_(1, 1), V_(1, 2), K_(1, 2),
                      V_(1, 3), V_(1, 4), V_(1, 5), K_(1, 3), V_(1, 6)]
        fillers[3] = [V_(1, 7), V_(1, 8), V_(1, 9), V_(1, 10), V_(1, 11),
                      V_(1, 12), V_(1, 13), V_(1, 14), V_(1, 15), Q_(1, 0),
                      Q_(1, 1)]
        fillers[4] = [Q_(1, 2), lambda: emit_og(0)] + OP_(0)
        fillers[5] = [Q_(1, 3), lambda: emit_og(1)] + OP_(1)
        fillers[6] = []
        fillers[7] = [lambda: emit_og(2)] + OP_(2)
        fillers[8] = [lambda: emit_og(3, final=True)] + OP_(3, final=True)

        def make_tail(u, s, hu, O_ps, act_assist=False):
            # normalize this half's O by the denominator row, stage for the
            # AllToAll; dest cores for the half's 512 tokens are 4s..4s+3.
            # On the final half, spread the copies onto the now-idle ACT
            # engine to shorten the exposed chain.
            def tail():
                for l in range(2):
                    obody = npool.tile([64, 512], bf16, tag=f"ob{l}",
                                       name=f"ob{hu}{l}")
                    if act_assist:
                        nc.scalar.activation(obody[:], O_ps[l][0:DH, :],
                                             AF.Identity)
                    else:
                        nc.vector.tensor_copy(obody[:], O_ps[l][0:DH, :])
                    recip = npool.tile([1, 512], f32, tag=f"rc{l}",
                                       name=f"rc{hu}{l}")
                    nc.vector.reciprocal(recip[:], O_ps[l][DH:DH + 1, :])
                    recipb = npool.tile([1, 512], bf16, tag=f"rb{l}",
                                        name=f"rb{hu}{l}")
                    nc.vector.tensor_copy(recipb[:], recip[:])
                    # broadcast across 64 partitions via a K=1 matmul
                    rB_ps = aux.tile([128, 512], f32, tag="aux",
                                     name=f"rB{hu}{l}")
                    nc.tensor.matmul(rB_ps[0:64, :], ones64_sb[:],
                                     recipb[:], start=True, stop=True,
                                     skip_group_check=True)
                    recipB = npool.tile([64, 512], bf16, tag=f"rB{l}",
                                        name=f"rB_{hu}{l}")
                    if act_assist:
                        nc.scalar.activation(recipB[:], rB_ps[0:64, :],
                                             AF.Identity)
                    else:
                        nc.vector.tensor_copy(recipB[:], rB_ps[0:64, :])
                    onorm = npool.tile([64, 512], bf16, tag=f"on{l}",
                                       name=f"on{hu}{l}")
                    nc.vector.tensor_mul(onorm[:], obody[:], recipB[:])
                    d = nc.sync.dma_start(
                        out=a2a_in[u][4 * s:4 * s + 4,
                                      64 * l:64 * (l + 1), :]
                        .rearrange("c p f -> p c f"),
                        in_=onorm[:].rearrange("p (c f) -> p c f", c=4))
                    _A2A_DMAS[u].append(d)
                if s == 1:
                    cc = nc.gpsimd.collective_compute(
                        "AllToAll", mybir.AluOpType.bypass,
                        replica_groups=[list(range(N_CORES))],
                        ins=[a2a_in[u][:].opt()], outs=[a2a_out[u][:].opt()])
                    # explicit edges: the AllToAll must not launch before
                    # every stage DMA (both halves) has LANDED
                    for d in _A2A_DMAS[u]:
                        _dep(cc.ins, d.ins, sync=True,
                             reason="a2a waits unit staging")
                    _CC[u] = cc
            return tail

        # ---- lead-in: minimum to start half 0 ----
        emit_qk(0, 0, wqT_sb, bq_f, QT_sb[0])
        emit_qk(0, 0, wkT_sb, bk_f, KT_sb[0])
        emit_v(0, 0)
        emit_v(0, 1)
        emit_v(0, 2)
        emit_v(0, 3)

        # ---- attention stream: 8 half-units ----
        pending_tail = None
        for hu in range(8):
            u, s = hu // 2, hu % 2
            b, h2 = u // 2, u % 2
            tq0 = 1024 * h2 + 512 * s
            fq = list(fillers[hu])
            if pending_tail is not None:
                fq.insert(0, pending_tail)

            O_ps = [att_o.tile([DH + 1, 512], f32, tag=f"o{l}",
                               name=f"O{hu}{l}") for l in range(2)]
            prevP = None
            for tk in range(16):
                s_ps = att_s.tile([128, 1024], f32, tag="s",
                                  name=f"S{hu}_{tk}")
                for l in range(2):
                    nc.tensor.matmul(
                        s_ps[:, 512 * l:512 * (l + 1)],
                        KT_sb[b][64 * l:64 * (l + 1),
                                 128 * tk:128 * (tk + 1)],
                        QT_sb[b][64 * l:64 * (l + 1), tq0:tq0 + 512],
                        start=True, stop=True)
                p_sb = pp.tile([128, 1024], bf16, tag="p",
                               name=f"P{hu}_{tk}")
                nc.scalar.activation(p_sb[:], s_ps[:], AF.Exp, scale=0.125)
                if tk > 0:
                    for l in range(2):
                        nc.tensor.matmul(
                            O_ps[l][:, :],
                            V_sb[b][tk - 1][:, l, :],
                            prevP[:, 512 * l:512 * (l + 1)],
                            start=(tk - 1 == 0), stop=False,
                            skip_group_check=True)
                prevP = p_sb
                if fq:
                    fq.pop(0)()
            for l in range(2):
                nc.tensor.matmul(
                    O_ps[l][:, :], V_sb[b][15][:, l, :],
                    prevP[:, 512 * l:512 * (l + 1)],
                    start=False, stop=True, skip_group_check=True)
            for f in fq:  # any leftover fillers
                f()
            if hu < 7:
                pending_tail = make_tail(u, s, hu, O_ps)
            else:
                make_tail(u, s, hu, O_ps, act_assist=True)()

        # ---- final epilogue: out-proj + LN of the last unit ----
        for f in fillers[8]:
            f()
        ctx.close()

    trace_sim = bool(os.environ.get("KERNEL_TRACE_SIM"))
    with tile.TileContext(nc, trace_sim=trace_sim) as tc:
        for rep in range(repeat):
            if rep:
                tc.strict_bb_all_engine_barrier()
            _emit_body(tc)

    nc.compile()
    return nc


def _prep_inputs(x, Wq, bq, Wk, bk, Wv, bv, Wo, bo, gamma, beta):
    x = np.asarray(x, dtype=np.float32)
    Wq, Wk, Wv, Wo = (np.asarray(a, dtype=np.float32) for a in (Wq, Wk, Wv, Wo))
    bq, bk, bv, bo = (np.asarray(a, dtype=np.float32) for a in (bq, bk, bv, bo))
    gamma = np.asarray(gamma, dtype=np.float32)
    beta = np.asarray(beta, dtype=np.float32)

    xT = [np.ascontiguousarray(x[b].T).astype(BF16) for b in range(B)]
    # gathered channel order (same for every core): src-core-major
    ch_gath = np.empty(D, dtype=np.int64)
    for g in range(D):
        i, jj, dd = g // 128, (g % 128) // 64, g % 64
        ch_gath[g] = 16 * dd + (2 * i + jj)
    woT = np.ascontiguousarray(Wo[:, ch_gath].T).astype(BF16)

    in_maps = []
    for c in range(N_CORES):
        ch_loc = np.empty(128, dtype=np.int64)
        for g in range(128):
            jj, dd = g // 64, g % 64
            ch_loc[g] = 16 * dd + (2 * c + jj)
        blob16 = np.empty(BF16_TOTAL, dtype=BF16)
        blob16[OFF_XT:OFF_WQ] = np.concatenate(
            [xT[0].reshape(-1), xT[1].reshape(-1)])
        blob16[OFF_WQ:OFF_WK] = Wq[ch_loc, :].T.reshape(-1).astype(BF16)
        blob16[OFF_WK:OFF_WV] = Wk[ch_loc, :].T.reshape(-1).astype(BF16)
        blob16[OFF_WV:OFF_WO] = Wv[ch_loc, :].T.reshape(-1).astype(BF16)
        blob16[OFF_WO:OFF_BQ] = woT.reshape(-1)
        blob16[OFF_BQ:OFF_BK] = bq[ch_loc].astype(BF16)
        blob16[OFF_BK:OFF_BV] = bk[ch_loc].astype(BF16)
        blob16[OFF_BV:OFF_GAMMA] = bv[ch_loc].astype(BF16)
        blob16[OFF_GAMMA:OFF_BETA] = gamma.astype(BF16)
        blob16[OFF_BETA:OFF_XRES] = beta.astype(BF16)
        # xres rows unit-major: row 128u + r = x[b(u), 1024*h2(u) + 128c + r]
        # with the out-proj bias folded in (y = proj + (x + bo)).
        xres = np.empty((TROWS, D), dtype=np.float32)
        for u in range(N_UNITS):
            bi, h2 = u // 2, u % 2
            t0 = 1024 * h2 + 128 * c
            xres[128 * u:128 * (u + 1)] = x[bi, t0:t0 + 128, :] + bo
        blob16[OFF_XRES:] = xres.reshape(-1).astype(BF16)
        in_maps.append({"blob16": blob16})
    return in_maps


def _run(in_maps):
    from concourse.bass_utils import run_bass_kernel_spmd
    if "nc" not in _CACHE:
        _CACHE["nc"] = _build()
    res = run_bass_kernel_spmd(_CACHE["nc"], in_maps,
                               core_ids=list(range(N_CORES)))
    return res


def kernel(x, Wq, bq, Wk, bk, Wv, bv, Wo, bo, gamma, beta):
    in_maps = _prep_inputs(x, Wq, bq, Wk, bk, Wv, bv, Wo, bo, gamma, beta)
    res = _run(in_maps)
    out = np.empty((B, T, D), dtype=np.float32)
    for c in range(N_CORES):
        r = np.asarray(res.results[c]["out"], dtype=np.float32)
        for u in range(N_UNITS):
            bi, h2 = u // 2, u % 2
            t0 = 1024 * h2 + 128 * c
            out[bi, t0:t0 + 128, :] = r[128 * u:128 * (u + 1), :]
    return out


# revision 16
# speedup vs baseline: 1.1355x; 1.1355x over previous
"""Fused MHA block (QKV + softmax + out-proj + residual + LayerNorm) for
Trainium2, SPMD over 8 NeuronCores — v2 "streaming" structure.

Sharding: head-parallel attention (core c owns heads {2c, 2c+1} for both
batches) with PER-UNIT output exchange. The 4 attention units are
(b, h2) = query blocks of 1024 tokens; each unit is processed as two
512-query half-units (s) so PSUM fits a double-buffered S ring. After each
unit, an 8-way AllToAll exchanges that unit's normalized head outputs so
core c receives ALL 1024 channels for its 128 tokens of the unit
(tokens [1024*h2 + 128*c, +128) of batch b); the unit's out-projection +
residual + LayerNorm then run as filler inside the next unit's exp chain.

Critical path = the ACT exp chain (16.8M exp elems/core, ~1.14us per
[128,1024] activation): everything else (QKV production, S/V matmuls,
collectives, out-proj, LN) is scheduled into its slack. Structure:
  - xT is streamed column-block-major so the first exp can issue ~5us in.
  - S^T layout (keys on partitions): softmax denominator = extra ones
    column in V (M=65 matmuls). exp scale 1/sqrt(64) folded into ACT.
  - S matmul pairs (K=64) land in distinct PE row-groups (tile_position
    auto-derived) -> concurrent on HW.
  - denominator broadcast via K=1 ones-matmul into the just-freed O bank.
  - LN rsqrt = exp(-0.5*ln(var+eps)): Ln+Exp live in the same ACT table
    set (natural_log_exp_and_others) -> no table swaps anywhere.
  - LN normalize runs on DVE (tensor_scalar), not ACT.
PSUM budget: S ring 2x[128,1024] (4 banks) + O/rb 2x[65,512] (2 banks) +
shared qk/v/out-proj ring 2x[128,512] (2 banks) = 8 banks exactly.
"""

import sys

sys.path.insert(0, "/opt/trn_rl_repo")

import numpy as np
import ml_dtypes

BF16 = ml_dtypes.bfloat16

B, T, D = 2, 2048, 1024
H, DH = 16, 64
N_CORES = 8
LN_EPS = 1e-5
HEADS_PER_CORE = 2
TROWS = T * B // N_CORES  # 512 output rows per core
NCH = 8  # 1024 / 128 contraction chunks
N_UNITS = 4  # (b, h2) query blocks

_CACHE = {}

# single-blob element offsets (everything bf16; per-input dispatch cost
# ~14us/tensor through the axon tunnel, so one arg is optimal)
OFF_XT = 0                              # 2 x [D, T]
OFF_WQ = OFF_XT + B * D * T
OFF_WK = OFF_WQ + D * 128
OFF_WV = OFF_WK + D * 128
OFF_WO = OFF_WV + D * 128
OFF_BQ = OFF_WO + D * D
OFF_BK = OFF_BQ + 128
OFF_BV = OFF_BK + 128
OFF_GAMMA = OFF_BV + 128
OFF_BETA = OFF_GAMMA + D
OFF_XRES = OFF_BETA + D                 # 512 rows, unit-major, bo folded in
BF16_TOTAL = OFF_XRES + TROWS * D


def _build(repeat=1, out_bf16=True):
    import os
    from contextlib import ExitStack
    import concourse.bass as bass
    import concourse.tile as tile
    from concourse import bacc, mybir

    import bass_rust
    _dep = bass_rust.add_dep_helper

    f32 = mybir.dt.float32
    bf16 = mybir.dt.bfloat16
    AF = mybir.ActivationFunctionType
    ALU = mybir.AluOpType

    def bcast(ap_src, parts):
        """Broadcast a 1-D (or row) AP across `parts` partitions (step 0)."""
        return bass.AP(tensor=ap_src.tensor, offset=ap_src.offset,
                       ap=[[0, parts]] + [list(p) for p in ap_src.ap])

    nc = bacc.Bacc("TRN2", target_bir_lowering=False, debug=False,
                   num_devices=N_CORES)

    b16 = nc.dram_tensor("blob16", [BF16_TOTAL], bf16, kind="ExternalInput")
    xT_d = [b16[OFF_XT + b * D * T:OFF_XT + (b + 1) * D * T]
            .rearrange("(c t) -> c t", t=T) for b in range(B)]
    wqT_d = b16[OFF_WQ:OFF_WQ + D * 128].rearrange("(c d) -> c d", d=128)
    wkT_d = b16[OFF_WK:OFF_WK + D * 128].rearrange("(c d) -> c d", d=128)
    wvT_d = b16[OFF_WV:OFF_WV + D * 128].rearrange("(c d) -> c d", d=128)
    woT_d = b16[OFF_WO:OFF_WO + D * D].rearrange("(c d) -> c d", d=D)
    bq_d = b16[OFF_BQ:OFF_BQ + 128]
    bk_d = b16[OFF_BK:OFF_BK + 128]
    bv_d = b16[OFF_BV:OFF_BV + 128]
    gamma_d = b16[OFF_GAMMA:OFF_GAMMA + D]
    beta_d = b16[OFF_BETA:OFF_BETA + D]
    xres_d = b16[OFF_XRES:OFF_XRES + TROWS * D].rearrange(
        "(r d) -> r d", d=D)
    out_dt = bf16 if out_bf16 else f32
    out_d = nc.dram_tensor("out", [TROWS, D], out_dt, kind="ExternalOutput")

    def _emit_body(tc):
        ctx = ExitStack()
        persist = ctx.enter_context(tc.tile_pool(name="persist", bufs=1))
        dram = ctx.enter_context(tc.tile_pool(name="dram", bufs=1,
                                              space="DRAM"))

        # ---- warm the natural_log_exp table set (Ln first narrows the
        # chosen set to one containing BOTH ln and exp; LN's rsqrt is
        # exp(-0.5*ln(v)) so no table swap ever happens) ----
        warm = persist.tile([1, 1], f32)
        nc.vector.memset(warm[:], 1.0)
        nc.scalar.activation(warm[:], warm[:], AF.Ln)
        nc.scalar.activation(warm[:], warm[:], AF.Exp, scale=0.125)

        # ---- persistent SBUF tiles ----
        xT_sb = [persist.tile([128, NCH, T], bf16, name=f"xT{b}sb")
                 for b in range(B)]
        wqT_sb = persist.tile([128, NCH, 128], bf16)
        wkT_sb = persist.tile([128, NCH, 128], bf16)
        wvT_sb = persist.tile([128, NCH, 128], bf16)
        bq_sb = persist.tile([128, 1], bf16)
        bk_sb = persist.tile([128, 1], bf16)
        bq_f = persist.tile([128, 1], f32)
        bk_f = persist.tile([128, 1], f32)
        bvb_sb = persist.tile([128, 128], bf16)  # bv bcast across partitions
        woT_sb = persist.tile([128, NCH, D], bf16)
        ones64_sb = persist.tile([1, 64], bf16)
        gamma_sb = persist.tile([128, D], bf16)
        beta_sb = persist.tile([128, D], bf16)
        eps_sb = persist.tile([128, 1], f32)
        xres_sb = persist.tile([128, N_UNITS, D], bf16)
        QT_sb = [persist.tile([128, T], bf16, name=f"QT{b}") for b in range(B)]
        KT_sb = [persist.tile([128, T], bf16, name=f"KT{b}") for b in range(B)]
        # V token-major with ones column per head: [t-tile][128, head, 65]
        V_sb = [[persist.tile([128, HEADS_PER_CORE, DH + 1], bf16,
                              name=f"V{b}_{tt}") for tt in range(16)]
                for b in range(B)]

        nc.vector.memset(ones64_sb[:], 1.0)
        nc.vector.memset(eps_sb[:], LN_EPS)
        for b in range(B):
            for tt in range(16):
                nc.vector.memset(V_sb[b][tt][:, :, DH:DH + 1], 1.0)

        # ---- DMA schedule ----
        # xT b0 n=0 column block first (gates the first S matmuls); rest of
        # b0 next; b1 during units 0-1. sync+scalar pre-start, sync+pool
        # after (scalar must stay free for the exp chain).
        def dma_xt(b, ci, n, eng):
            eng.dma_start(
                out=xT_sb[b][:, ci, 512 * n:512 * (n + 1)],
                in_=xT_d[b][128 * ci:128 * (ci + 1), 512 * n:512 * (n + 1)])

        for ci in range(NCH):
            dma_xt(0, ci, 0, nc.sync if ci % 2 == 0 else nc.scalar)
        nc.gpsimd.dma_start(out=bq_sb[:],
                            in_=bq_d.rearrange("(p f) -> p f", f=1))
        nc.gpsimd.dma_start(out=bk_sb[:],
                            in_=bk_d.rearrange("(p f) -> p f", f=1))
        nc.gpsimd.dma_start(out=wqT_sb[:],
                            in_=wqT_d.rearrange("(ci p) d -> p ci d", p=128))
        nc.gpsimd.dma_start(out=wkT_sb[:],
                            in_=wkT_d.rearrange("(ci p) d -> p ci d", p=128))
        nc.vector.tensor_copy(bq_f[:], bq_sb[:])
        nc.vector.tensor_copy(bk_f[:], bk_sb[:])
        nc.gpsimd.dma_start(out=wvT_sb[:],
                            in_=wvT_d.rearrange("(ci p) d -> p ci d", p=128))
        nc.gpsimd.dma_start(out=bvb_sb[:], in_=bcast(bv_d, 128))
        for n in range(1, 4):
            for ci in range(NCH):
                dma_xt(0, ci, n, nc.sync if ci % 2 == 0 else nc.scalar)
        for n in range(4):
            for ci in range(NCH):
                dma_xt(1, ci, n, nc.sync if ci % 2 == 0 else nc.gpsimd)
        for ci in range(NCH):
            nc.gpsimd.dma_start(out=woT_sb[:, ci, :],
                                in_=woT_d[128 * ci:128 * (ci + 1), :])
        nc.gpsimd.dma_start(out=gamma_sb[:], in_=bcast(gamma_d, 128))
        nc.gpsimd.dma_start(out=beta_sb[:], in_=bcast(beta_d, 128))
        for u in range(N_UNITS):
            nc.gpsimd.dma_start(out=xres_sb[:, u, :],
                                in_=xres_d[128 * u:128 * (u + 1), :])

        # ---- per-unit AllToAll DRAM buffers ----
        a2a_in = [dram.tile([N_CORES, 128, 128], bf16, name=f"a2ai{u}",
                            tag=f"ai{u}") for u in range(N_UNITS)]
        a2a_out = [dram.tile([N_CORES, 128, 128], bf16, name=f"a2ao{u}",
                             tag=f"ao{u}") for u in range(N_UNITS)]

        # ---- pools live for the whole attention+epilogue stream ----
        att_s = ctx.enter_context(tc.tile_pool(name="att_s", bufs=2,
                                               space="PSUM"))
        att_o = ctx.enter_context(tc.tile_pool(name="att_o", bufs=1,
                                               space="PSUM"))
        aux = ctx.enter_context(tc.tile_pool(name="aux", bufs=2,
                                             space="PSUM"))
        pp = ctx.enter_context(tc.tile_pool(name="pp", bufs=3))
        npool = ctx.enter_context(tc.tile_pool(name="npool", bufs=2))
        ogp = ctx.enter_context(tc.tile_pool(name="ogp", bufs=2))
        ln = ctx.enter_context(tc.tile_pool(name="ln", bufs=2))

        # ---- filler emitters (QKV production / out-proj+LN), called
        # between tk iterations so their PE/DVE work lands in exp slack ----
        def emit_qk(b, n, w_sb, bias_sb, dst, half=None, ps=None,
                    ci0=0, ci1=NCH):
            if ps is None:
                ps = aux.tile([128, 512], f32, tag="aux")
            for ci in range(ci0, ci1):
                nc.tensor.matmul(ps[:], w_sb[:, ci, :],
                                 xT_sb[b][:, ci, 512 * n:512 * (n + 1)],
                                 start=(ci == 0), stop=(ci == NCH - 1),
                                 skip_group_check=True)
            if ci1 == NCH:
                nc.vector.tensor_scalar(dst[:, 512 * n:512 * (n + 1)],
                                        ps[:], bias_sb[:], None, ALU.add)
            return ps

        def emit_v(b, tt):
            ps = aux.tile([128, 512], f32, tag="aux")
            for ci in range(NCH):
                nc.tensor.matmul(ps[:, 0:128],
                                 xT_sb[b][:, ci, 128 * tt:128 * (tt + 1)],
                                 wvT_sb[:, ci, :],
                                 start=(ci == 0), stop=(ci == NCH - 1))
            nc.vector.tensor_add(
                V_sb[b][tt][:, :, 0:DH],
                ps[:, 0:128].rearrange("p (h d) -> p h d", h=HEADS_PER_CORE),
                bvb_sb[:].rearrange("p (h d) -> p h d", h=HEADS_PER_CORE))

        def emit_proj_oc(u, oc, y_sb):
            og_sb = _OG[u]
            ps = aux.tile([128, 512], f32, tag="aux")
            for g in range(N_CORES):
                nc.tensor.matmul(ps[:], og_sb[:, g, :],
                                 woT_sb[:, g, 512 * oc:512 * (oc + 1)],
                                 start=(g == 0), stop=(g == N_CORES - 1),
                                 skip_group_check=True)
            nc.vector.tensor_add(
                y_sb[:, 512 * oc:512 * (oc + 1)], ps[:],
                xres_sb[:, u, 512 * oc:512 * (oc + 1)])
            stats = _STATS[u]
            nc.vector.bn_stats(out=stats[:, oc, :],
                               in_=y_sb[:, 512 * oc:512 * (oc + 1)])

        def emit_ln(u, y_sb, final=False):
            stats = _STATS[u]
            mv = ln.tile([128, nc.vector.BN_AGGR_DIM], f32, tag="mv")
            nc.vector.bn_aggr(out=mv[:], in_=stats[:])
            # rstd = exp(-0.5*ln(var+eps)) — same ACT table set as exp
            lnv = ln.tile([128, 1], f32, tag="lnv")
            nc.scalar.activation(lnv[:], mv[:, 1:2], AF.Ln, bias=eps_sb[:])
            rstd = ln.tile([128, 1], f32, tag="rstd")
            nc.scalar.activation(rstd[:], lnv[:], AF.Exp, scale=-0.5)
            negmr = ln.tile([128, 1], f32, tag="negmr")
            nc.vector.tensor_scalar(negmr[:], mv[:, 0:1], rstd[:], -1.0,
                                    ALU.mult, ALU.mult)
            yn = ln.tile([128, D], bf16, tag="yn")
            nc.vector.tensor_scalar(yn[:], y_sb[:], rstd[:], negmr[:],
                                    ALU.mult, ALU.add)
            fing = ln.tile([128, D], bf16, tag="fing")
            nc.vector.tensor_mul(fing[:], yn[:], gamma_sb[:])
            fin = ln.tile([128, D], out_dt, tag="fin")
            if final:
                nc.vector.tensor_add(fin[:], fing[:], beta_sb[:])
            else:
                nc.gpsimd.tensor_add(fin[:], fing[:], beta_sb[:])
            for oh in range(2):
                eng = nc.scalar if (final and oh == 1) else nc.sync
                eng.dma_start(
                    out=out_d[128 * u:128 * (u + 1),
                              512 * oh:512 * (oh + 1)],
                    in_=fin[:, 512 * oh:512 * (oh + 1)])

        _STATS = [None] * N_UNITS

        def OP_(u, final=False):
            state = {}

            def a():
                state["y"] = ln.tile([128, D], f32, tag="y", name=f"y{u}")
                _STATS[u] = ln.tile([128, 2, nc.vector.BN_STATS_DIM], f32,
                                    tag="stats", name=f"st{u}")
                emit_proj_oc(u, 0, state["y"])

            def bql():
                emit_proj_oc(u, 1, state["y"])

            def c():
                emit_ln(u, state["y"], final=final)
            return [a, bql, c]

        def emit_og(u, final=False):
            og_sb = ogp.tile([128, N_CORES, 128], bf16, tag="og",
                             name=f"og{u}")
            _OG[u] = og_sb
            engs = ((nc.sync, nc.scalar, nc.gpsimd) if final
                    else (nc.sync, nc.gpsimd))
            for i in range(N_CORES):
                d = engs[i % len(engs)].dma_start(out=og_sb[:, i, :],
                                                  in_=a2a_out[u][i])
                _dep(d.ins, _CC[u].ins, sync=True,
                     reason="og gather waits a2a")

        _OG = [None] * N_UNITS
        _A2A_DMAS = [[] for _ in range(N_UNITS)]
        _CC = [None] * N_UNITS

        # filler schedule: fillers[hu] = list of closures to interleave into
        # half-unit hu's tk loop (hu = 2*u + s, 8 halves total; [8] = final
        # epilogue). K(b,n) must complete before S(tk=4n); Q(b,n) before the
        # half that reads it; V(b,tt) only gates the O accumulation, so it
        # may lag. Each half's normalize tail is deferred into the next
        # half's first filler slot so it never stalls the exp chain.
        fillers = [[] for _ in range(9)]

        def K_(b, n):
            return lambda: emit_qk(b, n, wkT_sb, bk_f, KT_sb[b])

        def Q_(b, n):
            return lambda: emit_qk(b, n, wqT_sb, bq_f, QT_sb[b])

        def V_(b, tt):
            return lambda: emit_v(b, tt)

        # pops now happen at iterations 1..15 (tk0 is hoisted), so a
        # V(b,tt) filler must sit at slot <= tt-1.
        fillers[0] = [K_(0, 1), V_(0, 5), V_(0, 6), K_(0, 2), V_(0, 7),
                      V_(0, 8), V_(0, 9), K_(0, 3), V_(0, 10), V_(0, 11),
                      V_(0, 12), Q_(0, 1), V_(0, 13), V_(0, 14), V_(0, 15)]
        fillers[1] = [Q_(0, 2), Q_(0, 3), K_(1, 0)]
        fillers[2] = [K_(1, 1), V_(1, 0), V_(1, 1), V_(1, 2), K_(1, 2),
                      V_(1, 3), V_(1, 4), V_(1, 5), K_(1, 3), V_(1, 6)]
        fillers[3] = [V_(1, 7), V_(1, 8), V_(1, 9), V_(1, 10), V_(1, 11),
                      V_(1, 12), V_(1, 13), V_(1, 14), V_(1, 15), Q_(1, 0),
                      Q_(1, 1)]
        fillers[4] = [Q_(1, 2), lambda: emit_og(0)] + OP_(0)
        fillers[5] = [Q_(1, 3), lambda: emit_og(1)] + OP_(1)
        fillers[6] = []
        fillers[7] = [lambda: emit_og(2)] + OP_(2)
        fillers[8] = [lambda: emit_og(3, final=True)] + OP_(3, final=True)

        def make_tail(u, s, hu, O_ps, act_assist=False):
            # normalize this half's O by the denominator row, stage for the
            # AllToAll; dest cores for the half's 512 tokens are 4s..4s+3.
            # On the final half, spread the copies onto the now-idle ACT
            # engine to shorten the exposed chain.
            def tail():
                for l in range(2):
                    obody = npool.tile([64, 512], bf16, tag=f"ob{l}",
                                       name=f"ob{hu}{l}")
                    if act_assist:
                        nc.scalar.activation(obody[:], O_ps[l][0:DH, :],
                                             AF.Identity)
                    else:
                        nc.vector.tensor_copy(obody[:], O_ps[l][0:DH, :])
                    recip = npool.tile([1, 512], f32, tag=f"rc{l}",
                                       name=f"rc{hu}{l}")
                    nc.vector.reciprocal(recip[:], O_ps[l][DH:DH + 1, :])
                    recipb = npool.tile([1, 512], bf16, tag=f"rb{l}",
                                        name=f"rb{hu}{l}")
                    nc.vector.tensor_copy(recipb[:], recip[:])
                    # broadcast across 64 partitions via a K=1 matmul
                    rB_ps = aux.tile([128, 512], f32, tag="aux",
                                     name=f"rB{hu}{l}")
                    nc.tensor.matmul(rB_ps[0:64, :], ones64_sb[:],
                                     recipb[:], start=True, stop=True,
                                     skip_group_check=True)
                    recipB = npool.tile([64, 512], bf16, tag=f"rB{l}",
                                        name=f"rB_{hu}{l}")
                    if act_assist:
                        nc.scalar.activation(recipB[:], rB_ps[0:64, :],
                                             AF.Identity)
                    else:
                        nc.vector.tensor_copy(recipB[:], rB_ps[0:64, :])
                    onorm = npool.tile([64, 512], bf16, tag=f"on{l}",
                                       name=f"on{hu}{l}")
                    nc.vector.tensor_mul(onorm[:], obody[:], recipB[:])
                    d = nc.sync.dma_start(
                        out=a2a_in[u][4 * s:4 * s + 4,
                                      64 * l:64 * (l + 1), :]
                        .rearrange("c p f -> p c f"),
                        in_=onorm[:].rearrange("p (c f) -> p c f", c=4))
                    _A2A_DMAS[u].append(d)
                if s == 1:
                    cc = nc.gpsimd.collective_compute(
                        "AllToAll", mybir.AluOpType.bypass,
                        replica_groups=[list(range(N_CORES))],
                        ins=[a2a_in[u][:].opt()], outs=[a2a_out[u][:].opt()])
                    # explicit edges: the AllToAll must not launch before
                    # every stage DMA (both halves) has LANDED
                    for d in _A2A_DMAS[u]:
                        _dep(cc.ins, d.ins, sync=True,
                             reason="a2a waits unit staging")
                    _CC[u] = cc
            return tail

        # ---- lead-in: minimum to start half 0 ----
        emit_qk(0, 0, wqT_sb, bq_f, QT_sb[0])
        emit_qk(0, 0, wkT_sb, bk_f, KT_sb[0])
        emit_v(0, 0)
        emit_v(0, 1)
        emit_v(0, 2)
        emit_v(0, 3)
        emit_v(0, 4)

        def emit_s_exp(hu, tk):
            u, s = hu // 2, hu % 2
            b, h2 = u // 2, u % 2
            tq0 = 1024 * h2 + 512 * s
            s_ps = att_s.tile([128, 1024], f32, tag="s",
                              name=f"S{hu}_{tk}")
            for l in range(2):
                nc.tensor.matmul(
                    s_ps[:, 512 * l:512 * (l + 1)],
                    KT_sb[b][64 * l:64 * (l + 1),
                             128 * tk:128 * (tk + 1)],
                    QT_sb[b][64 * l:64 * (l + 1), tq0:tq0 + 512],
                    start=True, stop=True)
            p_sb = pp.tile([128, 1024], bf16, tag="p", name=f"P{hu}_{tk}")
            nc.scalar.activation(p_sb[:], s_ps[:], AF.Exp, scale=0.125)
            return p_sb

        # ---- attention stream: 8 half-units, software-pipelined so each
        # next half's first S+exp outranks leftover filler work ----
        pending_tail = None
        carryP = None
        for hu in range(8):
            u, s = hu // 2, hu % 2
            b = u // 2
            fq = list(fillers[hu])
            if pending_tail is not None:
                fq.insert(0, pending_tail)
                pending_tail = None

            O_ps = [att_o.tile([DH + 1, 512], f32, tag=f"o{l}",
                               name=f"O{hu}{l}") for l in range(2)]
            if carryP is None:
                prevP = emit_s_exp(hu, 0)
                start_tk = 1
            else:
                prevP = carryP
                carryP = None
                start_tk = 1
            for tk in range(start_tk, 16):
                p_sb = emit_s_exp(hu, tk)
                for l in range(2):
                    nc.tensor.matmul(
                        O_ps[l][:, :],
                        V_sb[b][tk - 1][:, l, :],
                        prevP[:, 512 * l:512 * (l + 1)],
                        start=(tk - 1 == 0), stop=False,
                        skip_group_check=True)
                prevP = p_sb
                if fq:
                    fq.pop(0)()
            for l in range(2):
                nc.tensor.matmul(
                    O_ps[l][:, :], V_sb[b][15][:, l, :],
                    prevP[:, 512 * l:512 * (l + 1)],
                    start=False, stop=True, skip_group_check=True)
            if hu < 7:
                # hoisted first iteration of the next half: highest PE/ACT
                # priority at the boundary, ahead of any filler backlog
                carryP = emit_s_exp(hu + 1, 0)
            for f in fq:  # any leftover fillers
                f()
            if hu < 7:
                pending_tail = make_tail(u, s, hu, O_ps)
            else:
                make_tail(u, s, hu, O_ps, act_assist=True)()
                # keep the PE array warm (HAM) while the final AllToAll
                # runs: a serially-chained trickle of tiny matmuls
                dummy_sb = persist.tile([1, 512], bf16, name="dummy")
                nc.vector.memset(dummy_sb[:], 1.0)
                for i in range(8):
                    dps = aux.tile([128, 512], f32, tag="aux",
                                   name=f"warmmm{i}")
                    nc.tensor.matmul(dps[0:64, :], ones64_sb[:],
                                     dummy_sb[:], start=True, stop=True,
                                     skip_group_check=True)
                    nc.vector.tensor_copy(dummy_sb[:, 0:8], dps[0:1, 0:8])

        # ---- final epilogue: out-proj + LN of the last unit ----
        for f in fillers[8]:
            f()
        ctx.close()

    trace_sim = bool(os.environ.get("KERNEL_TRACE_SIM"))
    with tile.TileContext(nc, trace_sim=trace_sim) as tc:
        for rep in range(repeat):
            if rep:
                tc.strict_bb_all_engine_barrier()
            _emit_body(tc)

    nc.compile()
    return nc


def _prep_inputs(x, Wq, bq, Wk, bk, Wv, bv, Wo, bo, gamma, beta):
    x = np.asarray(x, dtype=np.float32)
    Wq, Wk, Wv, Wo = (np.asarray(a, dtype=np.float32) for a in (Wq, Wk, Wv, Wo))
    bq, bk, bv, bo = (np.asarray(a, dtype=np.float32) for a in (bq, bk, bv, bo))
    gamma = np.asarray(gamma, dtype=np.float32)
    beta = np.asarray(beta, dtype=np.float32)

    xT = [np.ascontiguousarray(x[b].T).astype(BF16) for b in range(B)]
    # gathered channel order (same for every core): src-core-major
    ch_gath = np.empty(D, dtype=np.int64)
    for g in range(D):
        i, jj, dd = g // 128, (g % 128) // 64, g % 64
        ch_gath[g] = 16 * dd + (2 * i + jj)
    woT = np.ascontiguousarray(Wo[:, ch_gath].T).astype(BF16)

    in_maps = []
    for c in range(N_CORES):
        ch_loc = np.empty(128, dtype=np.int64)
        for g in range(128):
            jj, dd = g // 64, g % 64
            ch_loc[g] = 16 * dd + (2 * c + jj)
        blob16 = np.empty(BF16_TOTAL, dtype=BF16)
        blob16[OFF_XT:OFF_WQ] = np.concatenate(
            [xT[0].reshape(-1), xT[1].reshape(-1)])
        blob16[OFF_WQ:OFF_WK] = Wq[ch_loc, :].T.reshape(-1).astype(BF16)
        blob16[OFF_WK:OFF_WV] = Wk[ch_loc, :].T.reshape(-1).astype(BF16)
        blob16[OFF_WV:OFF_WO] = Wv[ch_loc, :].T.reshape(-1).astype(BF16)
        blob16[OFF_WO:OFF_BQ] = woT.reshape(-1)
        blob16[OFF_BQ:OFF_BK] = bq[ch_loc].astype(BF16)
        blob16[OFF_BK:OFF_BV] = bk[ch_loc].astype(BF16)
        blob16[OFF_BV:OFF_GAMMA] = bv[ch_loc].astype(BF16)
        blob16[OFF_GAMMA:OFF_BETA] = gamma.astype(BF16)
        blob16[OFF_BETA:OFF_XRES] = beta.astype(BF16)
        # xres rows unit-major: row 128u + r = x[b(u), 1024*h2(u) + 128c + r]
        # with the out-proj bias folded in (y = proj + (x + bo)).
        xres = np.empty((TROWS, D), dtype=np.float32)
        for u in range(N_UNITS):
            bi, h2 = u // 2, u % 2
            t0 = 1024 * h2 + 128 * c
            xres[128 * u:128 * (u + 1)] = x[bi, t0:t0 + 128, :] + bo
        blob16[OFF_XRES:] = xres.reshape(-1).astype(BF16)
        in_maps.append({"blob16": blob16})
    return in_maps


def _run(in_maps):
    from concourse.bass_utils import run_bass_kernel_spmd
    if "nc" not in _CACHE:
        _CACHE["nc"] = _build()
    res = run_bass_kernel_spmd(_CACHE["nc"], in_maps,
                               core_ids=list(range(N_CORES)))
    return res


def kernel(x, Wq, bq, Wk, bk, Wv, bv, Wo, bo, gamma, beta):
    in_maps = _prep_inputs(x, Wq, bq, Wk, bk, Wv, bv, Wo, bo, gamma, beta)
    res = _run(in_maps)
    out = np.empty((B, T, D), dtype=np.float32)
    for c in range(N_CORES):
        r = np.asarray(res.results[c]["out"], dtype=np.float32)
        for u in range(N_UNITS):
            bi, h2 = u // 2, u % 2
            t0 = 1024 * h2 + 128 * c
            out[bi, t0:t0 + 128, :] = r[128 * u:128 * (u + 1), :]
    return out


# revision 17
# speedup vs baseline: 1.2440x; 1.0956x over previous
"""Fused MHA block (QKV + softmax + out-proj + residual + LayerNorm) for
Trainium2, SPMD over 8 NeuronCores — v2 "streaming" structure.

Sharding: head-parallel attention (core c owns heads {2c, 2c+1} for both
batches) with PER-UNIT output exchange. The 4 attention units are
(b, h2) = query blocks of 1024 tokens; each unit is processed as two
512-query half-units (s) so PSUM fits a double-buffered S ring. After each
unit, an 8-way AllToAll exchanges that unit's normalized head outputs so
core c receives ALL 1024 channels for its 128 tokens of the unit
(tokens [1024*h2 + 128*c, +128) of batch b); the unit's out-projection +
residual + LayerNorm then run as filler inside the next unit's exp chain.

Critical path = the ACT exp chain (16.8M exp elems/core, ~1.14us per
[128,1024] activation): everything else (QKV production, S/V matmuls,
collectives, out-proj, LN) is scheduled into its slack. Structure:
  - xT is streamed column-block-major so the first exp can issue ~5us in.
  - S^T layout (keys on partitions): softmax denominator = extra ones
    column in V (M=65 matmuls). exp scale 1/sqrt(64) folded into ACT.
  - S matmul pairs (K=64) land in distinct PE row-groups (tile_position
    auto-derived) -> concurrent on HW.
  - denominator broadcast via K=1 ones-matmul into the just-freed O bank.
  - LN rsqrt = exp(-0.5*ln(var+eps)): Ln+Exp live in the same ACT table
    set (natural_log_exp_and_others) -> no table swaps anywhere.
  - LN normalize runs on DVE (tensor_scalar), not ACT.
PSUM budget: S ring 2x[128,1024] (4 banks) + O/rb 2x[65,512] (2 banks) +
shared qk/v/out-proj ring 2x[128,512] (2 banks) = 8 banks exactly.
"""

import sys

sys.path.insert(0, "/opt/trn_rl_repo")

import numpy as np
import ml_dtypes

BF16 = ml_dtypes.bfloat16

B, T, D = 2, 2048, 1024
H, DH = 16, 64
N_CORES = 8
LN_EPS = 1e-5
HEADS_PER_CORE = 2
TROWS = T * B // N_CORES  # 512 output rows per core
NCH = 8  # 1024 / 128 contraction chunks
N_UNITS = 4  # (b, h2) query blocks

_CACHE = {}

# single-blob element offsets (everything bf16; per-input dispatch cost
# ~14us/tensor through the axon tunnel, so one arg is optimal)
OFF_XT = 0                              # 2 x [D, T]
OFF_WQ = OFF_XT + B * D * T
OFF_WK = OFF_WQ + D * 128
OFF_WV = OFF_WK + D * 128
OFF_WO = OFF_WV + D * 128
OFF_BQ = OFF_WO + D * D
OFF_BK = OFF_BQ + 128
OFF_BV = OFF_BK + 128
OFF_GAMMA = OFF_BV + 128
OFF_BETA = OFF_GAMMA + D
OFF_XRES = OFF_BETA + D                 # 512 rows, unit-major, bo folded in
BF16_TOTAL = OFF_XRES + TROWS * D


def _build(repeat=1, out_bf16=True):
    import os
    from contextlib import ExitStack
    import concourse.bass as bass
    import concourse.tile as tile
    from concourse import bacc, mybir

    import bass_rust
    _dep = bass_rust.add_dep_helper

    f32 = mybir.dt.float32
    bf16 = mybir.dt.bfloat16
    AF = mybir.ActivationFunctionType
    ALU = mybir.AluOpType

    def bcast(ap_src, parts):
        """Broadcast a 1-D (or row) AP across `parts` partitions (step 0)."""
        return bass.AP(tensor=ap_src.tensor, offset=ap_src.offset,
                       ap=[[0, parts]] + [list(p) for p in ap_src.ap])

    nc = bacc.Bacc("TRN2", target_bir_lowering=False, debug=False,
                   num_devices=N_CORES)

    b16 = nc.dram_tensor("blob16", [BF16_TOTAL], bf16, kind="ExternalInput")
    xT_d = [b16[OFF_XT + b * D * T:OFF_XT + (b + 1) * D * T]
            .rearrange("(c t) -> c t", t=T) for b in range(B)]
    wqT_d = b16[OFF_WQ:OFF_WQ + D * 128].rearrange("(c d) -> c d", d=128)
    wkT_d = b16[OFF_WK:OFF_WK + D * 128].rearrange("(c d) -> c d", d=128)
    wvT_d = b16[OFF_WV:OFF_WV + D * 128].rearrange("(c d) -> c d", d=128)
    woT_d = b16[OFF_WO:OFF_WO + D * D].rearrange("(c d) -> c d", d=D)
    bq_d = b16[OFF_BQ:OFF_BQ + 128]
    bk_d = b16[OFF_BK:OFF_BK + 128]
    bv_d = b16[OFF_BV:OFF_BV + 128]
    gamma_d = b16[OFF_GAMMA:OFF_GAMMA + D]
    beta_d = b16[OFF_BETA:OFF_BETA + D]
    xres_d = b16[OFF_XRES:OFF_XRES + TROWS * D].rearrange(
        "(r d) -> r d", d=D)
    out_dt = bf16 if out_bf16 else f32
    out_d = nc.dram_tensor("out", [TROWS, D], out_dt, kind="ExternalOutput")

    def _emit_body(tc):
        ctx = ExitStack()
        persist = ctx.enter_context(tc.tile_pool(name="persist", bufs=1))
        dram = ctx.enter_context(tc.tile_pool(name="dram", bufs=1,
                                              space="DRAM"))

        # ---- warm the natural_log_exp table set (Ln first narrows the
        # chosen set to one containing BOTH ln and exp; LN's rsqrt is
        # exp(-0.5*ln(v)) so no table swap ever happens) ----
        warm = persist.tile([1, 1], f32)
        nc.vector.memset(warm[:], 1.0)
        nc.scalar.activation(warm[:], warm[:], AF.Ln)
        nc.scalar.activation(warm[:], warm[:], AF.Exp, scale=0.125)

        # ---- persistent SBUF tiles ----
        xT_sb = [persist.tile([128, NCH, T], bf16, name=f"xT{b}sb")
                 for b in range(B)]
        wqT_sb = persist.tile([128, NCH, 128], bf16)
        wkT_sb = persist.tile([128, NCH, 128], bf16)
        wvT_sb = persist.tile([128, NCH, 128], bf16)
        bq_sb = persist.tile([128, 1], bf16)
        bk_sb = persist.tile([128, 1], bf16)
        bq_f = persist.tile([128, 1], f32)
        bk_f = persist.tile([128, 1], f32)
        bvb_sb = persist.tile([128, 128], bf16)  # bv bcast across partitions
        woT_sb = persist.tile([128, NCH, D], bf16)
        ones64_sb = persist.tile([1, 64], bf16)
        gamma_sb = persist.tile([128, D], bf16)
        beta_sb = persist.tile([128, D], bf16)
        eps_sb = persist.tile([128, 1], f32)
        xres_sb = persist.tile([128, N_UNITS, D], bf16)
        QT_sb = [persist.tile([128, T], bf16, name=f"QT{b}") for b in range(B)]
        KT_sb = [persist.tile([128, T], bf16, name=f"KT{b}") for b in range(B)]
        # V token-major with ones column per head: [t-tile][128, head, 65]
        V_sb = [[persist.tile([128, HEADS_PER_CORE, DH + 1], bf16,
                              name=f"V{b}_{tt}") for tt in range(16)]
                for b in range(B)]

        nc.vector.memset(ones64_sb[:], 1.0)
        nc.vector.memset(eps_sb[:], LN_EPS)
        for b in range(B):
            for tt in range(16):
                nc.vector.memset(V_sb[b][tt][:, :, DH:DH + 1], 1.0)

        # ---- DMA schedule ----
        # xT b0 n=0 column block first (gates the first S matmuls); rest of
        # b0 next; b1 during units 0-1. sync+scalar pre-start, sync+pool
        # after (scalar must stay free for the exp chain).
        def dma_xt(b, ci, n, eng):
            eng.dma_start(
                out=xT_sb[b][:, ci, 512 * n:512 * (n + 1)],
                in_=xT_d[b][128 * ci:128 * (ci + 1), 512 * n:512 * (n + 1)])

        for ci in range(NCH):
            dma_xt(0, ci, 0, nc.sync if ci % 2 == 0 else nc.scalar)
        nc.gpsimd.dma_start(out=bq_sb[:],
                            in_=bq_d.rearrange("(p f) -> p f", f=1))
        nc.gpsimd.dma_start(out=bk_sb[:],
                            in_=bk_d.rearrange("(p f) -> p f", f=1))
        wq_r = wqT_d.rearrange("(ci p) d -> p ci d", p=128)
        nc.gpsimd.dma_start(out=wqT_sb[:, 0:4, :], in_=wq_r[:, 0:4, :])
        nc.gpsimd.dma_start(out=wqT_sb[:, 4:8, :], in_=wq_r[:, 4:8, :])
        nc.gpsimd.dma_start(out=wkT_sb[:],
                            in_=wkT_d.rearrange("(ci p) d -> p ci d", p=128))
        nc.vector.tensor_copy(bq_f[:], bq_sb[:])
        nc.vector.tensor_copy(bk_f[:], bk_sb[:])
        nc.gpsimd.dma_start(out=wvT_sb[:],
                            in_=wvT_d.rearrange("(ci p) d -> p ci d", p=128))
        nc.gpsimd.dma_start(out=bvb_sb[:], in_=bcast(bv_d, 128))
        for n in range(1, 4):
            for ci in range(NCH):
                dma_xt(0, ci, n, nc.sync if ci % 2 == 0 else nc.scalar)
        for n in range(4):
            for ci in range(NCH):
                dma_xt(1, ci, n, nc.sync if ci % 2 == 0 else nc.gpsimd)
        for ci in range(NCH):
            nc.gpsimd.dma_start(out=woT_sb[:, ci, :],
                                in_=woT_d[128 * ci:128 * (ci + 1), :])
        nc.gpsimd.dma_start(out=gamma_sb[:], in_=bcast(gamma_d, 128))
        nc.gpsimd.dma_start(out=beta_sb[:], in_=bcast(beta_d, 128))
        for u in range(N_UNITS):
            nc.gpsimd.dma_start(out=xres_sb[:, u, :],
                                in_=xres_d[128 * u:128 * (u + 1), :])

        # ---- per-unit AllToAll DRAM buffers ----
        a2a_in = [dram.tile([N_CORES, 128, 128], bf16, name=f"a2ai{u}",
                            tag=f"ai{u}") for u in range(N_UNITS)]
        a2a_out = [dram.tile([N_CORES, 128, 128], bf16, name=f"a2ao{u}",
                             tag=f"ao{u}") for u in range(N_UNITS)]

        # ---- pools live for the whole attention+epilogue stream ----
        att_s = ctx.enter_context(tc.tile_pool(name="att_s", bufs=2,
                                               space="PSUM"))
        att_o = ctx.enter_context(tc.tile_pool(name="att_o", bufs=1,
                                               space="PSUM"))
        aux = ctx.enter_context(tc.tile_pool(name="aux", bufs=2,
                                             space="PSUM"))
        pp = ctx.enter_context(tc.tile_pool(name="pp", bufs=3))
        npool = ctx.enter_context(tc.tile_pool(name="npool", bufs=2))
        ogp = ctx.enter_context(tc.tile_pool(name="ogp", bufs=2))
        ln = ctx.enter_context(tc.tile_pool(name="ln", bufs=2))

        # ---- filler emitters (QKV production / out-proj+LN), called
        # between tk iterations so their PE/DVE work lands in exp slack ----
        def emit_qk(b, n, w_sb, bias_sb, dst, half=None, ps=None,
                    ci0=0, ci1=NCH):
            if ps is None:
                ps = aux.tile([128, 512], f32, tag="aux")
            for ci in range(ci0, ci1):
                nc.tensor.matmul(ps[:], w_sb[:, ci, :],
                                 xT_sb[b][:, ci, 512 * n:512 * (n + 1)],
                                 start=(ci == 0), stop=(ci == NCH - 1),
                                 skip_group_check=True)
            if ci1 == NCH:
                nc.vector.tensor_scalar(dst[:, 512 * n:512 * (n + 1)],
                                        ps[:], bias_sb[:], None, ALU.add)
            return ps

        def emit_v(b, tt):
            ps = aux.tile([128, 512], f32, tag="aux")
            for ci in range(NCH):
                nc.tensor.matmul(ps[:, 0:128],
                                 xT_sb[b][:, ci, 128 * tt:128 * (tt + 1)],
                                 wvT_sb[:, ci, :],
                                 start=(ci == 0), stop=(ci == NCH - 1))
            nc.vector.tensor_add(
                V_sb[b][tt][:, :, 0:DH],
                ps[:, 0:128].rearrange("p (h d) -> p h d", h=HEADS_PER_CORE),
                bvb_sb[:].rearrange("p (h d) -> p h d", h=HEADS_PER_CORE))

        def emit_proj_oc(u, oc, y_sb):
            og_sb = _OG[u]
            ps = aux.tile([128, 512], f32, tag="aux")
            for g in range(N_CORES):
                nc.tensor.matmul(ps[:], og_sb[:, g, :],
                                 woT_sb[:, g, 512 * oc:512 * (oc + 1)],
                                 start=(g == 0), stop=(g == N_CORES - 1),
                                 skip_group_check=True)
            nc.vector.tensor_add(
                y_sb[:, 512 * oc:512 * (oc + 1)], ps[:],
                xres_sb[:, u, 512 * oc:512 * (oc + 1)])
            stats = _STATS[u]
            nc.vector.bn_stats(out=stats[:, oc, :],
                               in_=y_sb[:, 512 * oc:512 * (oc + 1)])

        def emit_ln(u, y_sb, final=False):
            stats = _STATS[u]
            mv = ln.tile([128, nc.vector.BN_AGGR_DIM], f32, tag="mv")
            nc.vector.bn_aggr(out=mv[:], in_=stats[:])
            # rstd = exp(-0.5*ln(var+eps)) — same ACT table set as exp
            lnv = ln.tile([128, 1], f32, tag="lnv")
            nc.scalar.activation(lnv[:], mv[:, 1:2], AF.Ln, bias=eps_sb[:])
            rstd = ln.tile([128, 1], f32, tag="rstd")
            nc.scalar.activation(rstd[:], lnv[:], AF.Exp, scale=-0.5)
            negmr = ln.tile([128, 1], f32, tag="negmr")
            nc.vector.tensor_scalar(negmr[:], mv[:, 0:1], rstd[:], -1.0,
                                    ALU.mult, ALU.mult)
            yn = ln.tile([128, D], bf16, tag="yn")
            nc.vector.tensor_scalar(yn[:], y_sb[:], rstd[:], negmr[:],
                                    ALU.mult, ALU.add)
            fing = ln.tile([128, D], bf16, tag="fing")
            nc.vector.tensor_mul(fing[:], yn[:], gamma_sb[:])
            fin = ln.tile([128, D], out_dt, tag="fin")
            if final:
                nc.vector.tensor_add(fin[:], fing[:], beta_sb[:])
            else:
                nc.gpsimd.tensor_add(fin[:], fing[:], beta_sb[:])
            for oh in range(2):
                eng = nc.scalar if (final and oh == 1) else nc.sync
                eng.dma_start(
                    out=out_d[128 * u:128 * (u + 1),
                              512 * oh:512 * (oh + 1)],
                    in_=fin[:, 512 * oh:512 * (oh + 1)])

        _STATS = [None] * N_UNITS

        def OP_(u, final=False):
            state = {}

            def a():
                state["y"] = ln.tile([128, D], f32, tag="y", name=f"y{u}")
                _STATS[u] = ln.tile([128, 2, nc.vector.BN_STATS_DIM], f32,
                                    tag="stats", name=f"st{u}")
                emit_proj_oc(u, 0, state["y"])

            def bql():
                emit_proj_oc(u, 1, state["y"])

            def c():
                emit_ln(u, state["y"], final=final)
            return [a, bql, c]

        def emit_og(u, final=False):
            og_sb = ogp.tile([128, N_CORES, 128], bf16, tag="og",
                             name=f"og{u}")
            _OG[u] = og_sb
            engs = ((nc.sync, nc.scalar, nc.gpsimd) if final
                    else (nc.sync, nc.gpsimd))
            for i in range(N_CORES):
                d = engs[i % len(engs)].dma_start(out=og_sb[:, i, :],
                                                  in_=a2a_out[u][i])
                _dep(d.ins, _CC[u].ins, sync=True,
                     reason="og gather waits a2a")

        _OG = [None] * N_UNITS
        _A2A_DMAS = [[] for _ in range(N_UNITS)]
        _CC = [None] * N_UNITS

        # filler schedule: fillers[hu] = list of closures to interleave into
        # half-unit hu's tk loop (hu = 2*u + s, 8 halves total; [8] = final
        # epilogue). K(b,n) must complete before S(tk=4n); Q(b,n) before the
        # half that reads it; V(b,tt) only gates the O accumulation, so it
        # may lag. Each half's normalize tail is deferred into the next
        # half's first filler slot so it never stalls the exp chain.
        fillers = [[] for _ in range(9)]

        # fillers emit in a low-priority band: the scheduler then treats
        # them as pure backfill — the attention stream's S/exp/V always
        # win a ready engine, and fillers run only in its slack (deps
        # still force completion before any consumer).
        LP_BAND = 1_000_000
        _lp = {"next": None}

        def lowprio(fn):
            def wrapped():
                orig = tc.cur_priority
                if _lp["next"] is None:
                    _lp["next"] = orig + LP_BAND
                tc.cur_priority = _lp["next"]
                fn()
                _lp["next"] = tc.cur_priority
                tc.cur_priority = orig
            return wrapped

        def K_(b, n):
            return lowprio(lambda: emit_qk(b, n, wkT_sb, bk_f, KT_sb[b]))

        def Q_(b, n):
            return lowprio(lambda: emit_qk(b, n, wqT_sb, bq_f, QT_sb[b]))

        def V_(b, tt):
            return lowprio(lambda: emit_v(b, tt))

        # pops now happen at iterations 1..15 (tk0 is hoisted), so a
        # V(b,tt) filler must sit at slot <= tt-1.
        fillers[0] = [K_(0, 1), V_(0, 5), V_(0, 6), K_(0, 2), V_(0, 7),
                      V_(0, 8), V_(0, 9), K_(0, 3), V_(0, 10), V_(0, 11),
                      V_(0, 12), Q_(0, 1), V_(0, 13), V_(0, 14), V_(0, 15)]
        fillers[1] = [Q_(0, 2), Q_(0, 3), K_(1, 0)]
        fillers[2] = [K_(1, 1), V_(1, 0), V_(1, 1), V_(1, 2), K_(1, 2),
                      V_(1, 3), V_(1, 4), V_(1, 5), K_(1, 3), V_(1, 6)]
        fillers[3] = [V_(1, 7), V_(1, 8), V_(1, 9), V_(1, 10), V_(1, 11),
                      V_(1, 12), V_(1, 13), V_(1, 14), V_(1, 15), Q_(1, 0),
                      Q_(1, 1)]
        fillers[4] = ([Q_(1, 2), lowprio(lambda: emit_og(0))]
                      + [lowprio(f) for f in OP_(0)])
        fillers[5] = ([Q_(1, 3), lowprio(lambda: emit_og(1))]
                      + [lowprio(f) for f in OP_(1)])
        fillers[6] = []
        fillers[7] = ([lowprio(lambda: emit_og(2))]
                      + [lowprio(f) for f in OP_(2)])
        fillers[8] = [lambda: emit_og(3, final=True)] + OP_(3, final=True)

        def make_tail(u, s, hu, O_ps, act_assist=False):
            # normalize this half's O by the denominator row, stage for the
            # AllToAll; dest cores for the half's 512 tokens are 4s..4s+3.
            # On the final half, spread the copies onto the now-idle ACT
            # engine to shorten the exposed chain.
            def tail():
                for l in range(2):
                    obody = npool.tile([64, 512], bf16, tag=f"ob{l}",
                                       name=f"ob{hu}{l}")
                    if act_assist:
                        nc.scalar.activation(obody[:], O_ps[l][0:DH, :],
                                             AF.Identity)
                    else:
                        nc.vector.tensor_copy(obody[:], O_ps[l][0:DH, :])
                    recip = npool.tile([1, 512], f32, tag=f"rc{l}",
                                       name=f"rc{hu}{l}")
                    nc.vector.reciprocal(recip[:], O_ps[l][DH:DH + 1, :])
                    recipb = npool.tile([1, 512], bf16, tag=f"rb{l}",
                                        name=f"rb{hu}{l}")
                    nc.vector.tensor_copy(recipb[:], recip[:])
                    # broadcast across 64 partitions via a K=1 matmul
                    rB_ps = aux.tile([128, 512], f32, tag="aux",
                                     name=f"rB{hu}{l}")
                    nc.tensor.matmul(rB_ps[0:64, :], ones64_sb[:],
                                     recipb[:], start=True, stop=True,
                                     skip_group_check=True)
                    recipB = npool.tile([64, 512], bf16, tag=f"rB{l}",
                                        name=f"rB_{hu}{l}")
                    if act_assist:
                        nc.scalar.activation(recipB[:], rB_ps[0:64, :],
                                             AF.Identity)
                    else:
                        nc.vector.tensor_copy(recipB[:], rB_ps[0:64, :])
                    onorm = npool.tile([64, 512], bf16, tag=f"on{l}",
                                       name=f"on{hu}{l}")
                    nc.vector.tensor_mul(onorm[:], obody[:], recipB[:])
                    d = nc.sync.dma_start(
                        out=a2a_in[u][4 * s:4 * s + 4,
                                      64 * l:64 * (l + 1), :]
                        .rearrange("c p f -> p c f"),
                        in_=onorm[:].rearrange("p (c f) -> p c f", c=4))
                    _A2A_DMAS[u].append(d)
                if s == 1:
                    cc = nc.gpsimd.collective_compute(
                        "AllToAll", mybir.AluOpType.bypass,
                        replica_groups=[list(range(N_CORES))],
                        ins=[a2a_in[u][:].opt()], outs=[a2a_out[u][:].opt()])
                    # explicit edges: the AllToAll must not launch before
                    # every stage DMA (both halves) has LANDED
                    for d in _A2A_DMAS[u]:
                        _dep(cc.ins, d.ins, sync=True,
                             reason="a2a waits unit staging")
                    _CC[u] = cc
            return tail

        # ---- lead-in: minimum to start half 0 ----
        emit_qk(0, 0, wqT_sb, bq_f, QT_sb[0])
        emit_qk(0, 0, wkT_sb, bk_f, KT_sb[0])
        emit_v(0, 0)
        emit_v(0, 1)
        emit_v(0, 2)
        emit_v(0, 3)
        emit_v(0, 4)

        def emit_s_exp(hu, tk):
            u, s = hu // 2, hu % 2
            b, h2 = u // 2, u % 2
            tq0 = 1024 * h2 + 512 * s
            s_ps = att_s.tile([128, 1024], f32, tag="s",
                              name=f"S{hu}_{tk}")
            for l in range(2):
                nc.tensor.matmul(
                    s_ps[:, 512 * l:512 * (l + 1)],
                    KT_sb[b][64 * l:64 * (l + 1),
                             128 * tk:128 * (tk + 1)],
                    QT_sb[b][64 * l:64 * (l + 1), tq0:tq0 + 512],
                    start=True, stop=True)
            p_sb = pp.tile([128, 1024], bf16, tag="p", name=f"P{hu}_{tk}")
            nc.scalar.activation(p_sb[:], s_ps[:], AF.Exp, scale=0.125)
            return p_sb

        # ---- attention stream: 8 half-units, software-pipelined so each
        # next half's first S+exp outranks leftover filler work ----
        pending_tail = None
        carryP = None
        for hu in range(8):
            u, s = hu // 2, hu % 2
            b = u // 2
            fq = list(fillers[hu])
            if pending_tail is not None:
                fq.insert(0, pending_tail)
                pending_tail = None

            O_ps = [att_o.tile([DH + 1, 512], f32, tag=f"o{l}",
                               name=f"O{hu}{l}") for l in range(2)]
            if carryP is None:
                prevP = emit_s_exp(hu, 0)
                start_tk = 1
            else:
                prevP = carryP
                carryP = None
                start_tk = 1
            for tk in range(start_tk, 16):
                p_sb = emit_s_exp(hu, tk)
                for l in range(2):
                    nc.tensor.matmul(
                        O_ps[l][:, :],
                        V_sb[b][tk - 1][:, l, :],
                        prevP[:, 512 * l:512 * (l + 1)],
                        start=(tk - 1 == 0), stop=False,
                        skip_group_check=True)
                prevP = p_sb
                if fq:
                    fq.pop(0)()
            for l in range(2):
                nc.tensor.matmul(
                    O_ps[l][:, :], V_sb[b][15][:, l, :],
                    prevP[:, 512 * l:512 * (l + 1)],
                    start=False, stop=True, skip_group_check=True)
            if hu < 7:
                # hoisted first iteration of the next half: highest PE/ACT
                # priority at the boundary, ahead of any filler backlog
                carryP = emit_s_exp(hu + 1, 0)
            for f in fq:  # any leftover fillers
                f()
            if hu < 7:
                pending_tail = make_tail(u, s, hu, O_ps)
            else:
                make_tail(u, s, hu, O_ps, act_assist=True)()
                # keep the PE array warm (HAM) while the final AllToAll
                # runs: a serially-chained trickle of tiny matmuls
                dummy_sb = persist.tile([1, 512], bf16, name="dummy")
                nc.vector.memset(dummy_sb[:], 1.0)
                for i in range(8):
                    dps = aux.tile([128, 512], f32, tag="aux",
                                   name=f"warmmm{i}")
                    nc.tensor.matmul(dps[0:64, :], ones64_sb[:],
                                     dummy_sb[:], start=True, stop=True,
                                     skip_group_check=True)
                    nc.vector.tensor_copy(dummy_sb[:, 0:8], dps[0:1, 0:8])

        # ---- final epilogue: out-proj + LN of the last unit ----
        for f in fillers[8]:
            f()
        ctx.close()

    trace_sim = bool(os.environ.get("KERNEL_TRACE_SIM"))
    with tile.TileContext(nc, trace_sim=trace_sim) as tc:
        for rep in range(repeat):
            if rep:
                tc.strict_bb_all_engine_barrier()
            _emit_body(tc)

    nc.compile()
    return nc


def _prep_inputs(x, Wq, bq, Wk, bk, Wv, bv, Wo, bo, gamma, beta):
    x = np.asarray(x, dtype=np.float32)
    Wq, Wk, Wv, Wo = (np.asarray(a, dtype=np.float32) for a in (Wq, Wk, Wv, Wo))
    bq, bk, bv, bo = (np.asarray(a, dtype=np.float32) for a in (bq, bk, bv, bo))
    gamma = np.asarray(gamma, dtype=np.float32)
    beta = np.asarray(beta, dtype=np.float32)

    xT = [np.ascontiguousarray(x[b].T).astype(BF16) for b in range(B)]
    # gathered channel order (same for every core): src-core-major
    ch_gath = np.empty(D, dtype=np.int64)
    for g in range(D):
        i, jj, dd = g // 128, (g % 128) // 64, g % 64
        ch_gath[g] = 16 * dd + (2 * i + jj)
    woT = np.ascontiguousarray(Wo[:, ch_gath].T).astype(BF16)

    in_maps = []
    for c in range(N_CORES):
        ch_loc = np.empty(128, dtype=np.int64)
        for g in range(128):
            jj, dd = g // 64, g % 64
            ch_loc[g] = 16 * dd + (2 * c + jj)
        blob16 = np.empty(BF16_TOTAL, dtype=BF16)
        blob16[OFF_XT:OFF_WQ] = np.concatenate(
            [xT[0].reshape(-1), xT[1].reshape(-1)])
        blob16[OFF_WQ:OFF_WK] = Wq[ch_loc, :].T.reshape(-1).astype(BF16)
        blob16[OFF_WK:OFF_WV] = Wk[ch_loc, :].T.reshape(-1).astype(BF16)
        blob16[OFF_WV:OFF_WO] = Wv[ch_loc, :].T.reshape(-1).astype(BF16)
        blob16[OFF_WO:OFF_BQ] = woT.reshape(-1)
        blob16[OFF_BQ:OFF_BK] = bq[ch_loc].astype(BF16)
        blob16[OFF_BK:OFF_BV] = bk[ch_loc].astype(BF16)
        blob16[OFF_BV:OFF_GAMMA] = bv[ch_loc].astype(BF16)
        blob16[OFF_GAMMA:OFF_BETA] = gamma.astype(BF16)
        blob16[OFF_BETA:OFF_XRES] = beta.astype(BF16)
        # xres rows unit-major: row 128u + r = x[b(u), 1024*h2(u) + 128c + r]
        # with the out-proj bias folded in (y = proj + (x + bo)).
        xres = np.empty((TROWS, D), dtype=np.float32)
        for u in range(N_UNITS):
            bi, h2 = u // 2, u % 2
            t0 = 1024 * h2 + 128 * c
            xres[128 * u:128 * (u + 1)] = x[bi, t0:t0 + 128, :] + bo
        blob16[OFF_XRES:] = xres.reshape(-1).astype(BF16)
        in_maps.append({"blob16": blob16})
    return in_maps


def _run(in_maps):
    from concourse.bass_utils import run_bass_kernel_spmd
    if "nc" not in _CACHE:
        _CACHE["nc"] = _build()
    res = run_bass_kernel_spmd(_CACHE["nc"], in_maps,
                               core_ids=list(range(N_CORES)))
    return res


def kernel(x, Wq, bq, Wk, bk, Wv, bv, Wo, bo, gamma, beta):
    in_maps = _prep_inputs(x, Wq, bq, Wk, bk, Wv, bv, Wo, bo, gamma, beta)
    res = _run(in_maps)
    out = np.empty((B, T, D), dtype=np.float32)
    for c in range(N_CORES):
        r = np.asarray(res.results[c]["out"], dtype=np.float32)
        for u in range(N_UNITS):
            bi, h2 = u // 2, u % 2
            t0 = 1024 * h2 + 128 * c
            out[bi, t0:t0 + 128, :] = r[128 * u:128 * (u + 1), :]
    return out
